# revision 9
# baseline (speedup 1.0000x reference)
"""Distributed multi-head attention block on 8 TRN2 NeuronCores.

Reference computation (B=2, S=2048, D=1024, H=16, DH=64):
    q = split_heads(q_ @ Wq + bq); k = ...; v = ...
    attn = softmax(q k^T / 8)  (mask is all-ones -> identity row mask)
    out = (merge_heads(attn @ v) + q_) @ Wf + bf

Sharding: 16 heads split 8 ways (2 heads / core); each core handles BOTH
batches.  The "virtual q" axis is b-major: vq = b*2048 + s (4096 total).

Per core c (heads 2c, 2c+1; d-dims 128c..128c+128):
  1. Projections (bf16): QT/KT [128 dh, 4096 vq], V [vk, 128 dh] from the
     transposed inputs xq/xk/xv [1024 din, 4096] and weight slices.
  2. Attention, transposed formulation: ST[k, q] = KT^T Q per (head, k-tile,
     q-chunk); exp via ScalarE straight from PSUM (scale=1/8 folded in);
     OT[dh, q] = V^T P accumulated over k-tiles; row-sums via ones-matmuls;
     softmax denominators broadcast with K=1 matmuls; normalize + residual
     on VectorE -> ZT_local [128 d, 4096 vq] (bf16).
  3. One 8-core AllToAll exchanges q-slices: core c ends with
     ZT_full [1024 d, 512] for virtual q chunk c, computes the final fc
     Y = ZT_full^T @ Wf (f32 out) for its 512 rows.

Host side: casts/transposes inputs (numpy), feeds per-core shards, places
each core's [512, 1024] output chunk, adds bf.  If the mask is not all-ones
(never happens with this problem's generator), falls back to a numpy
reference implementation.
"""

import sys

sys.path.insert(0, "/opt/trn_rl_repo")

import ml_dtypes
import numpy as np

import concourse.bass as bass
import concourse.tile as tile
from concourse import bacc, mybir
from concourse.bass_utils import run_bass_kernel_spmd

B, S, D, H = 2, 2048, 1024, 16
DH = D // H  # 64
N_CORES = 8
VQ = B * S  # 4096 virtual q (b-major)
NQC = VQ // 512  # 8 q-chunks of 512
NKT = S // 128  # 16 k-tiles per batch
NDIN = D // 128  # 8 din tiles

BF16 = mybir.dt.bfloat16
FP8 = mybir.dt.float8e4
F32 = mybir.dt.float32
AF = mybir.ActivationFunctionType
ALU = mybir.AluOpType
BF16NP = ml_dtypes.bfloat16
FP8NP = ml_dtypes.float8_e4m3
WSCALE = 32.0

_CACHE = {}


def _build():
    nc = bacc.Bacc(None, target_bir_lowering=False)

    xq = nc.declare_dram_parameter("xq", [D, VQ], FP8, isOutput=False)
    xk = nc.declare_dram_parameter("xk", [D, VQ], FP8, isOutput=False)
    xv = nc.declare_dram_parameter("xv", [D, VQ], FP8, isOutput=False)
    xres = nc.declare_dram_parameter("xres", [128, VQ], BF16, isOutput=False)
    wq = nc.declare_dram_parameter("wq", [D, 128], FP8, isOutput=False)
    wk = nc.declare_dram_parameter("wk", [D, 128], FP8, isOutput=False)
    wv = nc.declare_dram_parameter("wv", [D, 128], FP8, isOutput=False)
    wf = nc.declare_dram_parameter("wf", [D, D], BF16, isOutput=False)
    bq = nc.declare_dram_parameter("bq", [128, 1], F32, isOutput=False)
    bk = nc.declare_dram_parameter("bk", [128, 1], F32, isOutput=False)
    bv = nc.declare_dram_parameter("bv", [1, 128], BF16, isOutput=False)
    out = nc.declare_dram_parameter("out", [512, D], F32, isOutput=True)

    with tile.TileContext(nc) as tc:
        with (
            tc.tile_pool(name="persist", bufs=1) as sbp,
            tc.tile_pool(name="dram", bufs=1, space="DRAM") as dram,
        ):
            # ---- persistent SBUF tensors ----
            qt_sb = sbp.tile([128, VQ], BF16)  # [2 heads x 64 dh, vq]
            kt_sb = sbp.tile([128, VQ], BF16)  # [2 heads x 64 dh, vkey]
            v_sb = sbp.tile([128, 32 * 128], BF16)  # [k in tile, (b,kt) x 128 dh]
            wq_sb = sbp.tile([128, NDIN * 128], FP8)
            wk_sb = sbp.tile([128, NDIN * 128], FP8)
            wv_sb = sbp.tile([128, NDIN * 128], FP8)
            wf_sb = sbp.tile([128, NDIN * 1024], BF16)
            xres_sb = sbp.tile([128, VQ], BF16)
            zt_local = sbp.tile([128, VQ], BF16)
            zt_full = sbp.tile([128, NQC * 512], BF16)
            bq_sb = sbp.tile([128, 1], F32)
            bk_sb = sbp.tile([128, 1], F32)
            bv_sb = sbp.tile([1, 128], BF16)
            ones_col = sbp.tile([128, 1], BF16)
            ones_sb = sbp.tile([128, 128], BF16)
            ones32_sb = sbp.tile([128, 64], BF16)
            nc.vector.memset(ones_col[:], 1.0)
            nc.vector.memset(ones_sb[:], 1.0)
            nc.vector.memset(ones32_sb[:], WSCALE)

            # wv + biases needed first (V pass); wq/wk for the QT/KT passes;
            # wf/xres only needed at normalize/fc time -> issue on the scalar
            # engine's HWDGE so they never delay the x streams below.
            for j in range(NDIN):
                nc.sync.dma_start(wv_sb[:, 128 * j : 128 * (j + 1)], wv[128 * j : 128 * (j + 1), :])
            nc.sync.dma_start(bq_sb[:], bq[:])
            nc.sync.dma_start(bk_sb[:], bk[:])
            nc.sync.dma_start(bv_sb[:], bv[:])
            for j in range(NDIN):
                nc.sync.dma_start(wq_sb[:, 128 * j : 128 * (j + 1)], wq[128 * j : 128 * (j + 1), :])
                nc.sync.dma_start(wk_sb[:, 128 * j : 128 * (j + 1)], wk[128 * j : 128 * (j + 1), :])
            nc.scalar.dma_start(xres_sb[:], xres[:])
            for j in range(NDIN):
                nc.scalar.dma_start(wf_sb[:, 1024 * j : 1024 * (j + 1)], wf[128 * j : 128 * (j + 1), :])

            # =================== phase 1: projections ===================
            # order: xv first (V pass blocks attention via PSUM banks), then
            # xq -> QT, then xk -> KT (attention starts when KT lands).
            with (
                tc.tile_pool(name="xvp", bufs=8) as xvp,
                tc.tile_pool(name="xin", bufs=4) as xin,
                tc.tile_pool(name="ps1", bufs=8, space="PSUM") as ps1,
            ):
                # ---- V = xv^T @ wv  -> [vk, 128 dh], natural layout ----
                xv_tiles = []
                for din in range(NDIN):
                    xt = xvp.tile([128, VQ], FP8, name=f"xv{din}", tag="xv")
                    nc.sync.dma_start(xt[:], xv[128 * din : 128 * (din + 1), :])
                    xv_tiles.append(xt)
                for grp in range(4):  # 8 k-tiles per group
                    vps = []
                    for i in range(8):
                        vkt = grp * 8 + i
                        vp = ps1.tile([128, 512], F32, name=f"vps{vkt}", tag="ps")
                        vps.append(vp)
                        for din in range(NDIN):
                            nc.tensor.matmul(
                                vp[:, 0:128],
                                lhsT=xv_tiles[din][:, 128 * vkt : 128 * (vkt + 1)],
                                rhs=wv_sb[:, 128 * din : 128 * (din + 1)],
                                start=(din == 0),
                                stop=False,
                            )
                        # + bv broadcast over keys (rank-1)
                        nc.tensor.matmul(
                            vp[:, 0:128],
                            lhsT=ones_sb[0:1, :],
                            rhs=bv_sb[:],
                            start=False,
                            stop=True,
                        )
                    for i in range(8):
                        vkt = grp * 8 + i
                        nc.vector.tensor_copy(
                            v_sb[:, 128 * vkt : 128 * (vkt + 1)], vps[i][:, 0:128]
                        )

                # ---- QT = wq^T @ xq -> [128 dh, vq] ----
                for name, xdram, w_sb, b_sb, dst in (
                    ("q", xq, wq_sb, bq_sb, qt_sb),
                    ("k", xk, wk_sb, bk_sb, kt_sb),
                ):
                    pts = [
                        ps1.tile([128, 512], F32, name=f"{name}ps{qc}", tag="ps")
                        for qc in range(NQC)
                    ]
                    for din in range(NDIN):
                        xt = xin.tile([128, VQ], FP8, name=f"x{name}{din}", tag="x")
                        nc.sync.dma_start(xt[:], xdram[128 * din : 128 * (din + 1), :])
                        for qc in range(NQC):
                            nc.tensor.matmul(
                                pts[qc][:],
                                lhsT=w_sb[:, 128 * din : 128 * (din + 1)],
                                rhs=xt[:, 512 * qc : 512 * (qc + 1)],
                                start=(din == 0),
                                stop=(din == NDIN - 1),
                            )
                    for qc in range(NQC):
                        nc.vector.tensor_scalar_add(
                            dst[:, 512 * qc : 512 * (qc + 1)], pts[qc][:], b_sb[:]
                        )

            # =================== phase 2: attention ===================
            a2a_in = dram.tile([1024, 512], BF16)
            a2a_out = dram.tile([1024, 512], BF16)
            # Software pipeline by one q-chunk: while ScalarE exps chunk qc+1's
            # scores, the PE runs chunk qc's PV + rowsum matmuls (all of whose
            # inputs are ready) -> neither engine waits on the other.
            with (
                tc.tile_pool(name="stp", bufs=2, space="PSUM") as stp,  # 4 banks
                tc.tile_pool(name="otp", bufs=2, space="PSUM") as otp,  # 2 banks
                tc.tile_pool(name="rsp", bufs=2, space="PSUM") as rsp,  # 2 banks
                tc.tile_pool(name="ptp", bufs=20) as ptp,
                tc.tile_pool(name="nrm", bufs=3) as nrm,
            ):
                def emit_qk(qc, kt):
                    b = qc // 4
                    q0 = 512 * qc
                    kk = 2048 * b + 128 * kt
                    st = stp.tile([128, 1024], F32, name=f"st{qc}_{kt}", tag="st")
                    pt = ptp.tile([128, 1024], BF16, name=f"pt{qc}_{kt}", tag="pt")
                    for h in range(2):
                        nc.tensor.matmul(
                            st[:, 512 * h : 512 * (h + 1)],
                            lhsT=kt_sb[64 * h : 64 * (h + 1), kk : kk + 128],
                            rhs=qt_sb[64 * h : 64 * (h + 1), q0 : q0 + 512],
                            start=True,
                            stop=True,
                        )
                    nc.scalar.activation(pt[:], st[:], AF.Exp, scale=0.125 / (WSCALE * WSCALE))
                    return pt

                def emit_pv(qc, kt, ot, rs, pt):
                    b = qc // 4
                    vk = 128 * (16 * b + kt)
                    first = kt == 0
                    last = kt == NKT - 1
                    for h in range(2):
                        nc.tensor.matmul(
                            ot[64 * h : 64 * (h + 1), :],
                            lhsT=v_sb[:, vk + 64 * h : vk + 64 * (h + 1)],
                            rhs=pt[:, 512 * h : 512 * (h + 1)],
                            start=first,
                            stop=last,
                            tile_position=(0, 64 * h),
                        )
                    for h in range(2):
                        nc.tensor.matmul(
                            rs[32 * h : 32 * h + 1, :],
                            lhsT=ones_col[:],
                            rhs=pt[:, 512 * h : 512 * (h + 1)],
                            start=first,
                            stop=last,
                            tile_position=(0, 32 * h),
                        )

                def finish(qc, ot_sb, rs_bf):
                    # normalize + residual; PE part (bc) rides in the middle of
                    # the next round's matmul stream
                    q0 = 512 * qc
                    bc = rsp.tile([128, 512], F32, name=f"bc{qc}", tag="rs")
                    nc.tensor.matmul(
                        bc[0:64, :], lhsT=ones32_sb[0:1, 0:64], rhs=rs_bf[0:1, :],
                        start=True, stop=True, tile_position=(0, 0),
                    )
                    nc.tensor.matmul(
                        bc[64:128, :], lhsT=ones32_sb[32:33, 0:64], rhs=rs_bf[32:33, :],
                        start=True, stop=True, tile_position=(32, 64),
                    )
                    recipb = nrm.tile([128, 512], F32, name=f"recipb{qc}", tag="recipb")
                    nc.vector.reciprocal(recipb[:], bc[:])
                    o_tmp = nrm.tile([128, 512], BF16, name=f"otmp{qc}", tag="otmp")
                    nc.vector.tensor_tensor(o_tmp[:], ot_sb[:], recipb[:], ALU.mult)
                    nc.vector.tensor_tensor(
                        zt_local[:, q0 : q0 + 512], o_tmp[:], xres_sb[:, q0 : q0 + 512],
                        ALU.add,
                    )
                    nc.sync.dma_start(
                        a2a_in[128 * qc : 128 * (qc + 1), :],
                        zt_local[:, q0 : q0 + 512],
                    )

                pts = [emit_qk(0, kt) for kt in range(NKT)]  # prologue
                pending = None
                for qc in range(NQC):
                    ot = otp.tile([128, 512], F32, name=f"ot{qc}", tag="ot")
                    rs = rsp.tile([128, 512], F32, name=f"rs{qc}", tag="rs")
                    nxt = []
                    for kt in range(NKT):
                        emit_pv(qc, kt, ot, rs, pts[kt])
                        if qc + 1 < NQC:
                            nxt.append(emit_qk(qc + 1, kt))
                        if kt == 2 and pending is not None:
                            finish(*pending)
                            pending = None
                    pts = nxt
                    # drain psum accumulators to SBUF on VectorE so banks free
                    # without PE stalls
                    ot_sb = nrm.tile([128, 512], F32, name=f"otsb{qc}", tag="otsb")
                    nc.vector.tensor_copy(ot_sb[:], ot[:])
                    rs_bf = nrm.tile([128, 512], BF16, name=f"rsbf{qc}", tag="rsbf")
                    nc.vector.tensor_copy(rs_bf[0:1, :], rs[0:1, :])
                    nc.vector.tensor_copy(rs_bf[32:33, :], rs[32:33, :])
                    if pending is not None:
                        finish(*pending)
                    pending = (qc, ot_sb, rs_bf)
                finish(*pending)

            # =================== phase 3: A2A + fc ===================
            nc.gpsimd.collective_compute(
                "AllToAll",
                ALU.bypass,
                replica_groups=[list(range(N_CORES))],
                ins=[a2a_in.opt()],
                outs=[a2a_out.opt()],
            )
            for j in range(NQC):
                nc.sync.dma_start(
                    zt_full[:, 512 * j : 512 * (j + 1)], a2a_out[128 * j : 128 * (j + 1), :]
                )

            with (
                tc.tile_pool(name="fcps", bufs=4, space="PSUM") as fcps,
                tc.tile_pool(name="ysb", bufs=2) as ysb,
            ):
                for qt in range(4):
                    y = ysb.tile([128, 1024], F32, name=f"y{qt}", tag="y")
                    for nb in range(2):
                        yp = fcps.tile([128, 512], F32, name=f"yp{qt}_{nb}", tag="yp")
                        for j in range(NDIN):
                            nc.tensor.matmul(
                                yp[:],
                                lhsT=zt_full[:, 512 * j + 128 * qt : 512 * j + 128 * (qt + 1)],
                                rhs=wf_sb[:, 1024 * j + 512 * nb : 1024 * j + 512 * (nb + 1)],
                                start=(j == 0),
                                stop=(j == NDIN - 1),
                            )
                        nc.vector.tensor_copy(y[:, 512 * nb : 512 * (nb + 1)], yp[:])
                    nc.sync.dma_start(out[128 * qt : 128 * (qt + 1), :], y[:])

    nc.compile()
    return nc


def _numpy_reference(q_, k_, v_, mask, Wq, bq, Wk, bk, Wv, bv, Wf, bf):
    q_ = np.asarray(q_, np.float32)
    k_ = np.asarray(k_, np.float32)
    v_ = np.asarray(v_, np.float32)
    b = q_.shape[0]

    def split(x):
        return x.reshape(b, -1, H, DH).transpose(0, 2, 1, 3)

    q = split(q_ @ Wq + bq)
    k = split(k_ @ Wk + bk)
    v = split(v_ @ Wv + bv)
    attn = np.einsum("bhqd,bhkd->bhqk", q, k) / np.sqrt(np.float32(DH))
    attn = np.where(np.asarray(mask)[:, None, :, None], attn, np.float32(-1e12))
    attn = attn - attn.max(axis=-1, keepdims=True)
    e = np.exp(attn)
    p = e / e.sum(axis=-1, keepdims=True)
    o = np.einsum("bhqk,bhkd->bhqd", p, v)
    o = o.transpose(0, 2, 1, 3).reshape(b, -1, D)
    return (o + q_) @ Wf + bf


def kernel(q_, k_, v_, mask, Wq, bq, Wk, bk, Wv, bv, Wf, bf):
    mask = np.asarray(mask)
    if not mask.all():
        return _numpy_reference(q_, k_, v_, mask, Wq, bq, Wk, bk, Wv, bv, Wf, bf)

    q_ = np.asarray(q_, np.float32)
    k_ = np.asarray(k_, np.float32)
    v_ = np.asarray(v_, np.float32)

    # transposed, b-major-concatenated inputs (shared across cores)
    xq_f = np.ascontiguousarray(np.concatenate([q_[b].T for b in range(B)], axis=1))
    xq = xq_f.astype(FP8NP)
    xk = np.ascontiguousarray(np.concatenate([k_[b].T for b in range(B)], axis=1)).astype(FP8NP)
    xv = np.ascontiguousarray(np.concatenate([v_[b].T for b in range(B)], axis=1)).astype(FP8NP)
    wf_b = np.ascontiguousarray(np.asarray(Wf, np.float32)).astype(BF16NP)

    in_maps = []
    for c in range(N_CORES):
        d0 = 128 * c
        in_maps.append(
            {
                "xq": xq,
                "xk": xk,
                "xv": xv,
                "xres": np.ascontiguousarray(xq_f[d0 : d0 + 128, :]).astype(BF16NP),
                "wq": np.ascontiguousarray(np.asarray(Wq, np.float32)[:, d0 : d0 + 128] * WSCALE).astype(FP8NP),
                "wk": np.ascontiguousarray(np.asarray(Wk, np.float32)[:, d0 : d0 + 128] * WSCALE).astype(FP8NP),
                "wv": np.ascontiguousarray(np.asarray(Wv, np.float32)[:, d0 : d0 + 128] * WSCALE).astype(FP8NP),
                "wf": wf_b,
                "bq": np.ascontiguousarray(np.asarray(bq, np.float32)[d0 : d0 + 128, None] * WSCALE),
                "bk": np.ascontiguousarray(np.asarray(bk, np.float32)[d0 : d0 + 128, None] * WSCALE),
                "bv": np.ascontiguousarray(np.asarray(bv, np.float32)[None, d0 : d0 + 128] * WSCALE).astype(BF16NP),
            }
        )

    if "nc" not in _CACHE:
        _CACHE["nc"] = _build()
    res = run_bass_kernel_spmd(_CACHE["nc"], in_maps, core_ids=list(range(N_CORES)))

    out = np.empty((B, S, D), np.float32)
    for c in range(N_CORES):
        y = res.results[c]["out"]
        out[c // 4, 512 * (c % 4) : 512 * (c % 4 + 1), :] = y
    out += np.asarray(bf, np.float32)[None, None, :]
    return out


if __name__ == "__main__":
    # smoke test with small random data through the numpy fallback shapes
    rng = np.random.default_rng(0)
    args = dict(
        q_=rng.standard_normal((B, S, D), dtype=np.float32),
        k_=rng.standard_normal((B, S, D), dtype=np.float32),
        v_=rng.standard_normal((B, S, D), dtype=np.float32),
        mask=np.ones((B, S), bool),
        Wq=rng.standard_normal((D, D), dtype=np.float32) * 0.02,
        bq=np.zeros(D, np.float32),
        Wk=rng.standard_normal((D, D), dtype=np.float32) * 0.02,
        bk=np.zeros(D, np.float32),
        Wv=rng.standard_normal((D, D), dtype=np.float32) * 0.02,
        bv=np.zeros(D, np.float32),
        Wf=rng.standard_normal((D, D), dtype=np.float32) * 0.02,
        bf=np.zeros(D, np.float32),
    )
    got = kernel(**args)
    want = _numpy_reference(**args)
    rel = np.abs(got - want).max() / np.abs(want).max()
    print("rel_err:", rel)


# revision 10
# speedup vs baseline: 1.1055x; 1.1055x over previous
"""Distributed multi-head attention block on 8 TRN2 NeuronCores.

Reference computation (B=2, S=2048, D=1024, H=16, DH=64):
    q = split_heads(q_ @ Wq + bq); k = ...; v = ...
    attn = softmax(q k^T / 8)  (mask is all-ones -> identity row mask)
    out = (merge_heads(attn @ v) + q_) @ Wf + bf

Sharding: 16 heads split 8 ways (2 heads / core); each core handles BOTH
batches.  The "virtual q" axis is b-major: vq = b*2048 + s (4096 total).

Per core c (heads 2c, 2c+1; d-dims 128c..128c+128):
  1. Projections (bf16): QT/KT [128 dh, 4096 vq], V [vk, 128 dh] from the
     transposed inputs xq/xk/xv [1024 din, 4096] and weight slices.
  2. Attention, transposed formulation: ST[k, q] = KT^T Q per (head, k-tile,
     q-chunk); exp via ScalarE straight from PSUM (scale=1/8 folded in);
     OT[dh, q] = V^T P accumulated over k-tiles; row-sums via ones-matmuls;
     softmax denominators broadcast with K=1 matmuls; normalize + residual
     on VectorE -> ZT_local [128 d, 4096 vq] (bf16).
  3. One 8-core AllToAll exchanges q-slices: core c ends with
     ZT_full [1024 d, 512] for virtual q chunk c, computes the final fc
     Y = ZT_full^T @ Wf (f32 out) for its 512 rows.

Host side: casts/transposes inputs (numpy), feeds per-core shards, places
each core's [512, 1024] output chunk, adds bf.  If the mask is not all-ones
(never happens with this problem's generator), falls back to a numpy
reference implementation.
"""

import sys

sys.path.insert(0, "/opt/trn_rl_repo")

import ml_dtypes
import numpy as np

import concourse.bass as bass
import concourse.tile as tile
from concourse import bacc, mybir
from concourse.bass_utils import run_bass_kernel_spmd

B, S, D, H = 2, 2048, 1024, 16
DH = D // H  # 64
N_CORES = 8
VQ = B * S  # 4096 virtual q (b-major)
NQC = VQ // 512  # 8 q-chunks of 512
NKT = S // 128  # 16 k-tiles per batch
NDIN = D // 128  # 8 din tiles

BF16 = mybir.dt.bfloat16
FP8 = mybir.dt.float8e4
F32 = mybir.dt.float32
AF = mybir.ActivationFunctionType
ALU = mybir.AluOpType
BF16NP = ml_dtypes.bfloat16
FP8NP = ml_dtypes.float8_e4m3
WSCALE = 32.0

_CACHE = {}


def _build():
    nc = bacc.Bacc(None, target_bir_lowering=False)

    xq = nc.declare_dram_parameter("xq", [D, VQ], FP8, isOutput=False)
    xk = nc.declare_dram_parameter("xk", [D, VQ], FP8, isOutput=False)
    xv = nc.declare_dram_parameter("xv", [D, VQ], FP8, isOutput=False)
    wq = nc.declare_dram_parameter("wq", [D, 128], FP8, isOutput=False)
    wk = nc.declare_dram_parameter("wk", [D, 128], FP8, isOutput=False)
    wv = nc.declare_dram_parameter("wv", [D, 128], FP8, isOutput=False)
    wf = nc.declare_dram_parameter("wf", [D, D], BF16, isOutput=False)
    xresfc = nc.declare_dram_parameter("xresfc", [D, 512], BF16, isOutput=False)
    bq = nc.declare_dram_parameter("bq", [128, 1], F32, isOutput=False)
    bk = nc.declare_dram_parameter("bk", [128, 1], F32, isOutput=False)
    bv = nc.declare_dram_parameter("bv", [1, 128], BF16, isOutput=False)
    out = nc.declare_dram_parameter("out", [512, D], F32, isOutput=True)

    with tile.TileContext(nc) as tc:
        with (
            tc.tile_pool(name="persist", bufs=1) as sbp,
            tc.tile_pool(name="dram", bufs=1, space="DRAM") as dram,
        ):
            # ---- persistent SBUF tensors ----
            qt_sb = sbp.tile([128, VQ], BF16)  # [2 heads x 64 dh, vq]
            kt_sb = sbp.tile([128, VQ], BF16)  # [2 heads x 64 dh, vkey]
            v_sb = sbp.tile([128, 32 * 128], BF16)  # [k in tile, (b,kt) x 128 dh]
            wq_sb = sbp.tile([128, NDIN * 128], FP8)
            wk_sb = sbp.tile([128, NDIN * 128], FP8)
            wv_sb = sbp.tile([128, NDIN * 128], FP8)
            wf_sb = sbp.tile([128, NDIN * 1024], BF16)
            xresfc_sb = sbp.tile([128, NQC * 512], BF16)
            za_sb = sbp.tile([128, NQC * 512], FP8)
            zt_local = sbp.tile([128, VQ], FP8)
            zt_full = sbp.tile([128, NQC * 512], BF16)
            bq_sb = sbp.tile([128, 1], F32)
            bk_sb = sbp.tile([128, 1], F32)
            bv_sb = sbp.tile([1, 128], BF16)
            ones_col = sbp.tile([128, 1], BF16)
            ones_sb = sbp.tile([128, 128], BF16)
            nc.vector.memset(ones_col[:], 1.0)
            nc.vector.memset(ones_sb[:], 1.0)

            # wv + biases needed first (V pass); wq/wk for the QT/KT passes;
            # wf/xres only needed at normalize/fc time -> issue on the scalar
            # engine's HWDGE so they never delay the x streams below.
            for j in range(NDIN):
                nc.sync.dma_start(wv_sb[:, 128 * j : 128 * (j + 1)], wv[128 * j : 128 * (j + 1), :])
            nc.sync.dma_start(bq_sb[:], bq[:])
            nc.sync.dma_start(bk_sb[:], bk[:])
            nc.sync.dma_start(bv_sb[:], bv[:])
            for j in range(NDIN):
                nc.sync.dma_start(wq_sb[:, 128 * j : 128 * (j + 1)], wq[128 * j : 128 * (j + 1), :])
                nc.sync.dma_start(wk_sb[:, 128 * j : 128 * (j + 1)], wk[128 * j : 128 * (j + 1), :])
            for j in range(NQC):
                nc.scalar.dma_start(
                    xresfc_sb[:, 512 * j : 512 * (j + 1)], xresfc[128 * j : 128 * (j + 1), :]
                )
            for j in range(NDIN):
                nc.scalar.dma_start(wf_sb[:, 1024 * j : 1024 * (j + 1)], wf[128 * j : 128 * (j + 1), :])

            # =================== phase 1: projections ===================
            # order: xv first (V pass blocks attention via PSUM banks), then
            # xq -> QT, then xk -> KT (attention starts when KT lands).
            with (
                tc.tile_pool(name="xvp", bufs=8) as xvp,
                tc.tile_pool(name="xin", bufs=4) as xin,
                tc.tile_pool(name="ps1", bufs=8, space="PSUM") as ps1,
            ):
                # ---- V = xv^T @ wv  -> [vk, 128 dh], natural layout ----
                xv_tiles = []
                for din in range(NDIN):
                    xt = xvp.tile([128, VQ], FP8, name=f"xv{din}", tag="xv")
                    nc.sync.dma_start(xt[:], xv[128 * din : 128 * (din + 1), :])
                    xv_tiles.append(xt)
                for grp in range(4):  # 8 k-tiles per group
                    vps = []
                    for i in range(8):
                        vkt = grp * 8 + i
                        vp = ps1.tile([128, 512], F32, name=f"vps{vkt}", tag="ps")
                        vps.append(vp)
                        for din in range(NDIN):
                            nc.tensor.matmul(
                                vp[:, 0:128],
                                lhsT=xv_tiles[din][:, 128 * vkt : 128 * (vkt + 1)],
                                rhs=wv_sb[:, 128 * din : 128 * (din + 1)],
                                start=(din == 0),
                                stop=False,
                            )
                        # + bv broadcast over keys (rank-1)
                        nc.tensor.matmul(
                            vp[:, 0:128],
                            lhsT=ones_sb[0:1, :],
                            rhs=bv_sb[:],
                            start=False,
                            stop=True,
                        )
                    for i in range(8):
                        vkt = grp * 8 + i
                        nc.vector.tensor_copy(
                            v_sb[:, 128 * vkt : 128 * (vkt + 1)], vps[i][:, 0:128]
                        )

                # ---- QT = wq^T @ xq -> [128 dh, vq] ----
                for name, xdram, w_sb, b_sb, dst in (
                    ("q", xq, wq_sb, bq_sb, qt_sb),
                    ("k", xk, wk_sb, bk_sb, kt_sb),
                ):
                    pts = [
                        ps1.tile([128, 512], F32, name=f"{name}ps{qc}", tag="ps")
                        for qc in range(NQC)
                    ]
                    for din in range(NDIN):
                        xt = xin.tile([128, VQ], FP8, name=f"x{name}{din}", tag="x")
                        nc.sync.dma_start(xt[:], xdram[128 * din : 128 * (din + 1), :])
                        for qc in range(NQC):
                            nc.tensor.matmul(
                                pts[qc][:],
                                lhsT=w_sb[:, 128 * din : 128 * (din + 1)],
                                rhs=xt[:, 512 * qc : 512 * (qc + 1)],
                                start=(din == 0),
                                stop=(din == NDIN - 1),
                            )
                    for qc in range(NQC):
                        nc.vector.tensor_scalar_add(
                            dst[:, 512 * qc : 512 * (qc + 1)], pts[qc][:], b_sb[:]
                        )

            # =================== phase 2: attention ===================
            a2a_in = dram.tile([1024, 512], FP8)
            a2a_out = dram.tile([1024, 512], FP8)
            # Software pipeline by one q-chunk: while ScalarE exps chunk qc+1's
            # scores, the PE runs chunk qc's PV + rowsum matmuls (all of whose
            # inputs are ready) -> neither engine waits on the other.
            with (
                tc.tile_pool(name="stp", bufs=2, space="PSUM") as stp,  # 4 banks
                tc.tile_pool(name="otp", bufs=2, space="PSUM") as otp,  # 2 banks
                tc.tile_pool(name="rsp", bufs=2, space="PSUM") as rsp,  # 2 banks
                tc.tile_pool(name="ptp", bufs=20) as ptp,
                tc.tile_pool(name="nrm", bufs=3) as nrm,
            ):
                def emit_qk(qc, kt):
                    b = qc // 4
                    q0 = 512 * qc
                    kk = 2048 * b + 128 * kt
                    st = stp.tile([128, 1024], F32, name=f"st{qc}_{kt}", tag="st")
                    pt = ptp.tile([128, 1024], BF16, name=f"pt{qc}_{kt}", tag="pt")
                    for h in range(2):
                        nc.tensor.matmul(
                            st[:, 512 * h : 512 * (h + 1)],
                            lhsT=kt_sb[64 * h : 64 * (h + 1), kk : kk + 128],
                            rhs=qt_sb[64 * h : 64 * (h + 1), q0 : q0 + 512],
                            start=True,
                            stop=True,
                        )
                    nc.scalar.activation(pt[:], st[:], AF.Exp, scale=0.125 / (WSCALE * WSCALE))
                    return pt

                def emit_pv(qc, kt, ot, rs, pt):
                    b = qc // 4
                    vk = 128 * (16 * b + kt)
                    first = kt == 0
                    last = kt == NKT - 1
                    for h in range(2):
                        nc.tensor.matmul(
                            ot[64 * h : 64 * (h + 1), :],
                            lhsT=v_sb[:, vk + 64 * h : vk + 64 * (h + 1)],
                            rhs=pt[:, 512 * h : 512 * (h + 1)],
                            start=first,
                            stop=last,
                            tile_position=(0, 64 * h),
                        )
                    for h in range(2):
                        nc.tensor.matmul(
                            rs[32 * h : 32 * h + 1, :],
                            lhsT=ones_col[:],
                            rhs=pt[:, 512 * h : 512 * (h + 1)],
                            start=first,
                            stop=last,
                            tile_position=(0, 32 * h),
                        )

                def finish(qc, ot_sb, rs_bf):
                    # normalize + residual; PE part (bc) rides in the middle of
                    # the next round's matmul stream
                    q0 = 512 * qc
                    bc = rsp.tile([128, 512], F32, name=f"bc{qc}", tag="rs")
                    nc.tensor.matmul(
                        bc[0:64, :], lhsT=ones_sb[0:1, 0:64], rhs=rs_bf[0:1, :],
                        start=True, stop=True, tile_position=(0, 0),
                    )
                    nc.tensor.matmul(
                        bc[64:128, :], lhsT=ones_sb[32:33, 0:64], rhs=rs_bf[32:33, :],
                        start=True, stop=True, tile_position=(32, 64),
                    )
                    recipb = nrm.tile([128, 512], F32, name=f"recipb{qc}", tag="recipb")
                    nc.vector.reciprocal(recipb[:], bc[:])
                    nc.vector.tensor_tensor(
                        zt_local[:, q0 : q0 + 512], ot_sb[:], recipb[:], ALU.mult
                    )
                    nc.sync.dma_start(
                        a2a_in[128 * qc : 128 * (qc + 1), :],
                        zt_local[:, q0 : q0 + 512],
                    )

                pts = [emit_qk(0, kt) for kt in range(NKT)]  # prologue
                pending = None
                for qc in range(NQC):
                    ot = otp.tile([128, 512], F32, name=f"ot{qc}", tag="ot")
                    rs = rsp.tile([128, 512], F32, name=f"rs{qc}", tag="rs")
                    nxt = []
                    for kt in range(NKT):
                        emit_pv(qc, kt, ot, rs, pts[kt])
                        if qc + 1 < NQC:
                            nxt.append(emit_qk(qc + 1, kt))
                        if kt == 2 and pending is not None:
                            finish(*pending)
                            pending = None
                    pts = nxt
                    # drain psum accumulators to SBUF on VectorE so banks free
                    # without PE stalls
                    ot_sb = nrm.tile([128, 512], F32, name=f"otsb{qc}", tag="otsb")
                    nc.vector.tensor_copy(ot_sb[:], ot[:])
                    rs_bf = nrm.tile([128, 512], BF16, name=f"rsbf{qc}", tag="rsbf")
                    nc.vector.tensor_copy(rs_bf[0:1, :], rs[0:1, :])
                    nc.vector.tensor_copy(rs_bf[32:33, :], rs[32:33, :])
                    if pending is not None:
                        finish(*pending)
                    pending = (qc, ot_sb, rs_bf)
                finish(*pending)

            # =================== phase 3: A2A + fc ===================
            nc.gpsimd.collective_compute(
                "AllToAll",
                ALU.bypass,
                replica_groups=[list(range(N_CORES))],
                ins=[a2a_in.opt()],
                outs=[a2a_out.opt()],
            )
            for j in range(NQC):
                nc.sync.dma_start(
                    za_sb[:, 512 * j : 512 * (j + 1)], a2a_out[128 * j : 128 * (j + 1), :]
                )
            for j in range(NQC):
                nc.vector.scalar_tensor_tensor(
                    zt_full[:, 512 * j : 512 * (j + 1)],
                    za_sb[:, 512 * j : 512 * (j + 1)],
                    1.0 / WSCALE,
                    xresfc_sb[:, 512 * j : 512 * (j + 1)],
                    ALU.mult,
                    ALU.add,
                )

            with (
                tc.tile_pool(name="fcps", bufs=4, space="PSUM") as fcps,
                tc.tile_pool(name="ysb", bufs=2) as ysb,
            ):
                for qt in range(4):
                    y = ysb.tile([128, 1024], F32, name=f"y{qt}", tag="y")
                    for nb in range(2):
                        yp = fcps.tile([128, 512], F32, name=f"yp{qt}_{nb}", tag="yp")
                        for j in range(NDIN):
                            nc.tensor.matmul(
                                yp[:],
                                lhsT=zt_full[:, 512 * j + 128 * qt : 512 * j + 128 * (qt + 1)],
                                rhs=wf_sb[:, 1024 * j + 512 * nb : 1024 * j + 512 * (nb + 1)],
                                start=(j == 0),
                                stop=(j == NDIN - 1),
                            )
                        nc.vector.tensor_copy(y[:, 512 * nb : 512 * (nb + 1)], yp[:])
                    nc.sync.dma_start(out[128 * qt : 128 * (qt + 1), :], y[:])

    nc.compile()
    return nc


def _numpy_reference(q_, k_, v_, mask, Wq, bq, Wk, bk, Wv, bv, Wf, bf):
    q_ = np.asarray(q_, np.float32)
    k_ = np.asarray(k_, np.float32)
    v_ = np.asarray(v_, np.float32)
    b = q_.shape[0]

    def split(x):
        return x.reshape(b, -1, H, DH).transpose(0, 2, 1, 3)

    q = split(q_ @ Wq + bq)
    k = split(k_ @ Wk + bk)
    v = split(v_ @ Wv + bv)
    attn = np.einsum("bhqd,bhkd->bhqk", q, k) / np.sqrt(np.float32(DH))
    attn = np.where(np.asarray(mask)[:, None, :, None], attn, np.float32(-1e12))
    attn = attn - attn.max(axis=-1, keepdims=True)
    e = np.exp(attn)
    p = e / e.sum(axis=-1, keepdims=True)
    o = np.einsum("bhqk,bhkd->bhqd", p, v)
    o = o.transpose(0, 2, 1, 3).reshape(b, -1, D)
    return (o + q_) @ Wf + bf


def kernel(q_, k_, v_, mask, Wq, bq, Wk, bk, Wv, bv, Wf, bf):
    mask = np.asarray(mask)
    if not mask.all():
        return _numpy_reference(q_, k_, v_, mask, Wq, bq, Wk, bk, Wv, bv, Wf, bf)

    q_ = np.asarray(q_, np.float32)
    k_ = np.asarray(k_, np.float32)
    v_ = np.asarray(v_, np.float32)

    # transposed, b-major-concatenated inputs (shared across cores)
    xq_f = np.ascontiguousarray(np.concatenate([q_[b].T for b in range(B)], axis=1))
    xq = xq_f.astype(FP8NP)
    xk = np.ascontiguousarray(np.concatenate([k_[b].T for b in range(B)], axis=1)).astype(FP8NP)
    xv = np.ascontiguousarray(np.concatenate([v_[b].T for b in range(B)], axis=1)).astype(FP8NP)
    wf_b = np.ascontiguousarray(np.asarray(Wf, np.float32)).astype(BF16NP)

    in_maps = []
    for c in range(N_CORES):
        d0 = 128 * c
        in_maps.append(
            {
                "xq": xq,
                "xk": xk,
                "xv": xv,
                "xresfc": np.ascontiguousarray(
                    q_[c // 4].T[:, 512 * (c % 4) : 512 * (c % 4 + 1)]
                ).astype(BF16NP),
                "wq": np.ascontiguousarray(np.asarray(Wq, np.float32)[:, d0 : d0 + 128] * WSCALE).astype(FP8NP),
                "wk": np.ascontiguousarray(np.asarray(Wk, np.float32)[:, d0 : d0 + 128] * WSCALE).astype(FP8NP),
                "wv": np.ascontiguousarray(np.asarray(Wv, np.float32)[:, d0 : d0 + 128] * WSCALE).astype(FP8NP),
                "wf": wf_b,
                "bq": np.ascontiguousarray(np.asarray(bq, np.float32)[d0 : d0 + 128, None] * WSCALE),
                "bk": np.ascontiguousarray(np.asarray(bk, np.float32)[d0 : d0 + 128, None] * WSCALE),
                "bv": np.ascontiguousarray(np.asarray(bv, np.float32)[None, d0 : d0 + 128] * WSCALE).astype(BF16NP),
            }
        )

    if "nc" not in _CACHE:
        _CACHE["nc"] = _build()
    res = run_bass_kernel_spmd(_CACHE["nc"], in_maps, core_ids=list(range(N_CORES)))

    out = np.empty((B, S, D), np.float32)
    for c in range(N_CORES):
        y = res.results[c]["out"]
        out[c // 4, 512 * (c % 4) : 512 * (c % 4 + 1), :] = y
    out += np.asarray(bf, np.float32)[None, None, :]
    return out


if __name__ == "__main__":
    # smoke test with small random data through the numpy fallback shapes
    rng = np.random.default_rng(0)
    args = dict(
        q_=rng.standard_normal((B, S, D), dtype=np.float32),
        k_=rng.standard_normal((B, S, D), dtype=np.float32),
        v_=rng.standard_normal((B, S, D), dtype=np.float32),
        mask=np.ones((B, S), bool),
        Wq=rng.standard_normal((D, D), dtype=np.float32) * 0.02,
        bq=np.zeros(D, np.float32),
        Wk=rng.standard_normal((D, D), dtype=np.float32) * 0.02,
        bk=np.zeros(D, np.float32),
        Wv=rng.standard_normal((D, D), dtype=np.float32) * 0.02,
        bv=np.zeros(D, np.float32),
        Wf=rng.standard_normal((D, D), dtype=np.float32) * 0.02,
        bf=np.zeros(D, np.float32),
    )
    got = kernel(**args)
    want = _numpy_reference(**args)
    rel = np.abs(got - want).max() / np.abs(want).max()
    print("rel_err:", rel)


# revision 11
# speedup vs baseline: 1.1829x; 1.0700x over previous
"""Distributed multi-head attention block on 8 TRN2 NeuronCores.

Reference computation (B=2, S=2048, D=1024, H=16, DH=64):
    q = split_heads(q_ @ Wq + bq); k = ...; v = ...
    attn = softmax(q k^T / 8)  (mask is all-ones -> identity row mask)
    out = (merge_heads(attn @ v) + q_) @ Wf + bf

Sharding: 16 heads split 8 ways (2 heads / core); each core handles BOTH
batches.  The "virtual q" axis is b-major: vq = b*2048 + s (4096 total).

Per core c (heads 2c, 2c+1; d-dims 128c..128c+128):
  1. Projections (bf16): QT/KT [128 dh, 4096 vq], V [vk, 128 dh] from the
     transposed inputs xq/xk/xv [1024 din, 4096] and weight slices.
  2. Attention, transposed formulation: ST[k, q] = KT^T Q per (head, k-tile,
     q-chunk); exp via ScalarE straight from PSUM (scale=1/8 folded in);
     OT[dh, q] = V^T P accumulated over k-tiles; row-sums via ones-matmuls;
     softmax denominators broadcast with K=1 matmuls; normalize + residual
     on VectorE -> ZT_local [128 d, 4096 vq] (bf16).
  3. One 8-core AllToAll exchanges q-slices: core c ends with
     ZT_full [1024 d, 512] for virtual q chunk c, computes the final fc
     Y = ZT_full^T @ Wf (f32 out) for its 512 rows.

Host side: casts/transposes inputs (numpy), feeds per-core shards, places
each core's [512, 1024] output chunk, adds bf.  If the mask is not all-ones
(never happens with this problem's generator), falls back to a numpy
reference implementation.
"""

import sys

sys.path.insert(0, "/opt/trn_rl_repo")

import ml_dtypes
import numpy as np

import concourse.bass as bass
import concourse.tile as tile
from concourse import bacc, mybir
from concourse.bass_utils import run_bass_kernel_spmd

B, S, D, H = 2, 2048, 1024, 16
DH = D // H  # 64
N_CORES = 8
VQ = B * S  # 4096 virtual q (b-major)
NQC = VQ // 512  # 8 q-chunks of 512
NKT = S // 128  # 16 k-tiles per batch
NDIN = D // 128  # 8 din tiles

BF16 = mybir.dt.bfloat16
FP8 = mybir.dt.float8e4
F32 = mybir.dt.float32
AF = mybir.ActivationFunctionType
ALU = mybir.AluOpType
BF16NP = ml_dtypes.bfloat16
FP8NP = ml_dtypes.float8_e4m3
WSCALE = 32.0

_CACHE = {}


def _build():
    nc = bacc.Bacc(None, target_bir_lowering=False)

    xq = nc.declare_dram_parameter("xq", [D, VQ], FP8, isOutput=False)
    xk = nc.declare_dram_parameter("xk", [D, VQ], FP8, isOutput=False)
    xv = nc.declare_dram_parameter("xv", [D, VQ], FP8, isOutput=False)
    wq = nc.declare_dram_parameter("wq", [D, 128], FP8, isOutput=False)
    wk = nc.declare_dram_parameter("wk", [D, 128], FP8, isOutput=False)
    wv = nc.declare_dram_parameter("wv", [D, 128], FP8, isOutput=False)
    wf = nc.declare_dram_parameter("wf", [D, D], BF16, isOutput=False)
    xresfc = nc.declare_dram_parameter("xresfc", [D, 512], BF16, isOutput=False)
    bq = nc.declare_dram_parameter("bq", [128, 1], F32, isOutput=False)
    bk = nc.declare_dram_parameter("bk", [128, 1], F32, isOutput=False)
    bv = nc.declare_dram_parameter("bv", [1, 128], BF16, isOutput=False)
    out = nc.declare_dram_parameter("out", [512, D], F32, isOutput=True)

    with tile.TileContext(nc) as tc:
        with (
            tc.tile_pool(name="persist", bufs=1) as sbp,
            tc.tile_pool(name="dram", bufs=1, space="DRAM") as dram,
        ):
            # ---- persistent SBUF tensors ----
            qt_sb = sbp.tile([128, VQ], BF16)  # [2 heads x 64 dh, vq]
            kt_sb = sbp.tile([128, VQ], BF16)  # [2 heads x 64 dh, vkey]
            v_sb = sbp.tile([128, 32 * 128], BF16)  # [k in tile, (b,kt) x 128 dh]
            wq_sb = sbp.tile([128, NDIN * 128], FP8)
            wk_sb = sbp.tile([128, NDIN * 128], FP8)
            wv_sb = sbp.tile([128, NDIN * 128], FP8)
            wf_sb = sbp.tile([128, NDIN * 1024], BF16)
            xresfc_sb = sbp.tile([128, NQC * 512], BF16)
            za_sb = sbp.tile([128, NQC * 512], FP8)
            zt_local = sbp.tile([128, VQ], FP8)
            zt_full = sbp.tile([128, NQC * 512], BF16)
            bq_sb = sbp.tile([128, 1], F32)
            bk_sb = sbp.tile([128, 1], F32)
            bv_sb = sbp.tile([1, 128], BF16)
            ones_col = sbp.tile([128, 1], BF16)
            ones_sb = sbp.tile([128, 128], BF16)
            nc.vector.memset(ones_col[:], 1.0)
            nc.vector.memset(ones_sb[:], 1.0)

            # wv + biases needed first (V pass); wq/wk for the QT/KT passes;
            # wf/xres only needed at normalize/fc time -> issue on the scalar
            # engine's HWDGE so they never delay the x streams below.
            nc.sync.dma_start(
                wv_sb[:].rearrange("p (t n) -> p t n", t=NDIN),
                wv[:].rearrange("(t p) n -> p t n", p=128),
            )
            nc.sync.dma_start(bq_sb[:], bq[:])
            nc.sync.dma_start(bk_sb[:], bk[:])
            nc.sync.dma_start(bv_sb[:], bv[:])
            nc.sync.dma_start(
                wq_sb[:].rearrange("p (t n) -> p t n", t=NDIN),
                wq[:].rearrange("(t p) n -> p t n", p=128),
            )
            nc.sync.dma_start(
                wk_sb[:].rearrange("p (t n) -> p t n", t=NDIN),
                wk[:].rearrange("(t p) n -> p t n", p=128),
            )
            nc.scalar.dma_start(
                xresfc_sb[:].rearrange("p (t n) -> p t n", t=NQC),
                xresfc[:].rearrange("(t p) n -> p t n", p=128),
            )
            nc.scalar.dma_start(
                wf_sb[:].rearrange("p (t n) -> p t n", t=NDIN),
                wf[:].rearrange("(t p) n -> p t n", p=128),
            )

            # =================== phase 1: projections ===================
            # order: xv first (V pass blocks attention via PSUM banks), then
            # xq -> QT, then xk -> KT (attention starts when KT lands).
            with (
                tc.tile_pool(name="xvp", bufs=1) as xvp,
                tc.tile_pool(name="xin", bufs=3) as xin,
                tc.tile_pool(name="ps1", bufs=8, space="PSUM") as ps1,
            ):
                # ---- V = xv^T @ wv  -> [vk, 128 dh], natural layout ----
                xv_all = xvp.tile([128, NDIN * VQ], FP8, name="xv_all", tag="xv")
                nc.sync.dma_start(
                    xv_all[:].rearrange("p (t v) -> p t v", t=NDIN),
                    xv[:].rearrange("(t p) v -> p t v", p=128),
                )
                for grp in range(4):  # 8 k-tiles per group
                    vps = []
                    for i in range(8):
                        vkt = grp * 8 + i
                        vp = ps1.tile([128, 512], F32, name=f"vps{vkt}", tag="ps")
                        vps.append(vp)
                        for din in range(NDIN):
                            nc.tensor.matmul(
                                vp[:, 0:128],
                                lhsT=xv_all[:, VQ * din + 128 * vkt : VQ * din + 128 * (vkt + 1)],
                                rhs=wv_sb[:, 128 * din : 128 * (din + 1)],
                                start=(din == 0),
                                stop=False,
                            )
                        # + bv broadcast over keys (rank-1)
                        nc.tensor.matmul(
                            vp[:, 0:128],
                            lhsT=ones_sb[0:1, :],
                            rhs=bv_sb[:],
                            start=False,
                            stop=True,
                        )
                    for i in range(8):
                        vkt = grp * 8 + i
                        nc.vector.tensor_copy(
                            v_sb[:, 128 * vkt : 128 * (vkt + 1)], vps[i][:, 0:128]
                        )

                # ---- QT = wq^T @ xq -> [128 dh, vq] ----
                for name, xdram, w_sb, b_sb, dst in (
                    ("q", xq, wq_sb, bq_sb, qt_sb),
                    ("k", xk, wk_sb, bk_sb, kt_sb),
                ):
                    pts = [
                        ps1.tile([128, 512], F32, name=f"{name}ps{qc}", tag="ps")
                        for qc in range(NQC)
                    ]
                    for half in range(2):
                        xt = xin.tile([128, 4 * VQ], FP8, name=f"x{name}{half}", tag="x")
                        nc.sync.dma_start(
                            xt[:].rearrange("p (t v) -> p t v", t=4),
                            xdram[512 * half : 512 * (half + 1), :].rearrange(
                                "(t p) v -> p t v", p=128
                            ),
                        )
                        for dj in range(4):
                            din = 4 * half + dj
                            for qc in range(NQC):
                                nc.tensor.matmul(
                                    pts[qc][:],
                                    lhsT=w_sb[:, 128 * din : 128 * (din + 1)],
                                    rhs=xt[:, VQ * dj + 512 * qc : VQ * dj + 512 * (qc + 1)],
                                    start=(din == 0),
                                    stop=(din == NDIN - 1),
                                )
                    for qc in range(NQC):
                        nc.vector.tensor_scalar_add(
                            dst[:, 512 * qc : 512 * (qc + 1)], pts[qc][:], b_sb[:]
                        )

            # =================== phase 2: attention ===================
            a2a_in = dram.tile([1024, 512], FP8)
            a2a_out = dram.tile([1024, 512], FP8)
            # Software pipeline by one q-chunk: while ScalarE exps chunk qc+1's
            # scores, the PE runs chunk qc's PV + rowsum matmuls (all of whose
            # inputs are ready) -> neither engine waits on the other.
            with (
                tc.tile_pool(name="stp", bufs=2, space="PSUM") as stp,  # 4 banks
                tc.tile_pool(name="otp", bufs=2, space="PSUM") as otp,  # 2 banks
                tc.tile_pool(name="rsp", bufs=2, space="PSUM") as rsp,  # 2 banks
                tc.tile_pool(name="ptp", bufs=20) as ptp,
                tc.tile_pool(name="nrm", bufs=3) as nrm,
            ):
                def emit_qk(qc, kt):
                    b = qc // 4
                    q0 = 512 * qc
                    kk = 2048 * b + 128 * kt
                    st = stp.tile([128, 1024], F32, name=f"st{qc}_{kt}", tag="st")
                    pt = ptp.tile([128, 1024], BF16, name=f"pt{qc}_{kt}", tag="pt")
                    for h in range(2):
                        nc.tensor.matmul(
                            st[:, 512 * h : 512 * (h + 1)],
                            lhsT=kt_sb[64 * h : 64 * (h + 1), kk : kk + 128],
                            rhs=qt_sb[64 * h : 64 * (h + 1), q0 : q0 + 512],
                            start=True,
                            stop=True,
                        )
                    nc.scalar.activation(pt[:], st[:], AF.Exp, scale=0.125 / (WSCALE * WSCALE))
                    return pt

                def emit_pv(qc, kt, ot, rs, pt):
                    b = qc // 4
                    vk = 128 * (16 * b + kt)
                    first = kt == 0
                    last = kt == NKT - 1
                    for h in range(2):
                        nc.tensor.matmul(
                            ot[64 * h : 64 * (h + 1), :],
                            lhsT=v_sb[:, vk + 64 * h : vk + 64 * (h + 1)],
                            rhs=pt[:, 512 * h : 512 * (h + 1)],
                            start=first,
                            stop=last,
                            tile_position=(0, 64 * h),
                        )
                    for h in range(2):
                        nc.tensor.matmul(
                            rs[32 * h : 32 * h + 1, :],
                            lhsT=ones_col[:],
                            rhs=pt[:, 512 * h : 512 * (h + 1)],
                            start=first,
                            stop=last,
                            tile_position=(0, 32 * h),
                        )

                def finish(qc, ot_sb, rs_bf):
                    # normalize + residual; PE part (bc) rides in the middle of
                    # the next round's matmul stream
                    q0 = 512 * qc
                    bc = rsp.tile([128, 512], F32, name=f"bc{qc}", tag="rs")
                    nc.tensor.matmul(
                        bc[0:64, :], lhsT=ones_sb[0:1, 0:64], rhs=rs_bf[0:1, :],
                        start=True, stop=True, tile_position=(0, 0),
                    )
                    nc.tensor.matmul(
                        bc[64:128, :], lhsT=ones_sb[32:33, 0:64], rhs=rs_bf[32:33, :],
                        start=True, stop=True, tile_position=(32, 64),
                    )
                    recipb = nrm.tile([128, 512], F32, name=f"recipb{qc}", tag="recipb")
                    nc.vector.reciprocal(recipb[:], bc[:])
                    nc.vector.tensor_tensor(
                        zt_local[:, q0 : q0 + 512], ot_sb[:], recipb[:], ALU.mult
                    )
                    nc.sync.dma_start(
                        a2a_in[128 * qc : 128 * (qc + 1), :],
                        zt_local[:, q0 : q0 + 512],
                    )

                pts = [emit_qk(0, kt) for kt in range(NKT)]  # prologue
                pending = None
                for qc in range(NQC):
                    ot = otp.tile([128, 512], F32, name=f"ot{qc}", tag="ot")
                    rs = rsp.tile([128, 512], F32, name=f"rs{qc}", tag="rs")
                    nxt = []
                    for kt in range(NKT):
                        emit_pv(qc, kt, ot, rs, pts[kt])
                        if qc + 1 < NQC:
                            nxt.append(emit_qk(qc + 1, kt))
                        if kt == 2 and pending is not None:
                            finish(*pending)
                            pending = None
                    pts = nxt
                    # drain psum accumulators to SBUF on VectorE so banks free
                    # without PE stalls
                    ot_sb = nrm.tile([128, 512], F32, name=f"otsb{qc}", tag="otsb")
                    nc.vector.tensor_copy(ot_sb[:], ot[:])
                    rs_bf = nrm.tile([128, 512], BF16, name=f"rsbf{qc}", tag="rsbf")
                    nc.vector.tensor_copy(rs_bf[0:1, :], rs[0:1, :])
                    nc.vector.tensor_copy(rs_bf[32:33, :], rs[32:33, :])
                    if pending is not None:
                        finish(*pending)
                    pending = (qc, ot_sb, rs_bf)
                finish(*pending)

            # =================== phase 3: A2A + fc ===================
            nc.gpsimd.collective_compute(
                "AllToAll",
                ALU.bypass,
                replica_groups=[list(range(N_CORES))],
                ins=[a2a_in.opt()],
                outs=[a2a_out.opt()],
            )
            nc.sync.dma_start(
                za_sb[:].rearrange("p (t v) -> p t v", t=NQC),
                a2a_out[:].rearrange("(t p) v -> p t v", p=128),
            )
            for j in range(NQC):
                nc.vector.scalar_tensor_tensor(
                    zt_full[:, 512 * j : 512 * (j + 1)],
                    za_sb[:, 512 * j : 512 * (j + 1)],
                    1.0 / WSCALE,
                    xresfc_sb[:, 512 * j : 512 * (j + 1)],
                    ALU.mult,
                    ALU.add,
                )

            with (
                tc.tile_pool(name="fcps", bufs=4, space="PSUM") as fcps,
                tc.tile_pool(name="ysb", bufs=2) as ysb,
            ):
                for qt in range(4):
                    y = ysb.tile([128, 1024], F32, name=f"y{qt}", tag="y")
                    for nb in range(2):
                        yp = fcps.tile([128, 512], F32, name=f"yp{qt}_{nb}", tag="yp")
                        for j in range(NDIN):
                            nc.tensor.matmul(
                                yp[:],
                                lhsT=zt_full[:, 512 * j + 128 * qt : 512 * j + 128 * (qt + 1)],
                                rhs=wf_sb[:, 1024 * j + 512 * nb : 1024 * j + 512 * (nb + 1)],
                                start=(j == 0),
                                stop=(j == NDIN - 1),
                            )
                        nc.vector.tensor_copy(y[:, 512 * nb : 512 * (nb + 1)], yp[:])
                    nc.sync.dma_start(out[128 * qt : 128 * (qt + 1), :], y[:])

    nc.compile()
    return nc


def _numpy_reference(q_, k_, v_, mask, Wq, bq, Wk, bk, Wv, bv, Wf, bf):
    q_ = np.asarray(q_, np.float32)
    k_ = np.asarray(k_, np.float32)
    v_ = np.asarray(v_, np.float32)
    b = q_.shape[0]

    def split(x):
        return x.reshape(b, -1, H, DH).transpose(0, 2, 1, 3)

    q = split(q_ @ Wq + bq)
    k = split(k_ @ Wk + bk)
    v = split(v_ @ Wv + bv)
    attn = np.einsum("bhqd,bhkd->bhqk", q, k) / np.sqrt(np.float32(DH))
    attn = np.where(np.asarray(mask)[:, None, :, None], attn, np.float32(-1e12))
    attn = attn - attn.max(axis=-1, keepdims=True)
    e = np.exp(attn)
    p = e / e.sum(axis=-1, keepdims=True)
    o = np.einsum("bhqk,bhkd->bhqd", p, v)
    o = o.transpose(0, 2, 1, 3).reshape(b, -1, D)
    return (o + q_) @ Wf + bf


def kernel(q_, k_, v_, mask, Wq, bq, Wk, bk, Wv, bv, Wf, bf):
    mask = np.asarray(mask)
    if not mask.all():
        return _numpy_reference(q_, k_, v_, mask, Wq, bq, Wk, bk, Wv, bv, Wf, bf)

    q_ = np.asarray(q_, np.float32)
    k_ = np.asarray(k_, np.float32)
    v_ = np.asarray(v_, np.float32)

    # transposed, b-major-concatenated inputs (shared across cores)
    xq_f = np.ascontiguousarray(np.concatenate([q_[b].T for b in range(B)], axis=1))
    xq = xq_f.astype(FP8NP)
    xk = np.ascontiguousarray(np.concatenate([k_[b].T for b in range(B)], axis=1)).astype(FP8NP)
    xv = np.ascontiguousarray(np.concatenate([v_[b].T for b in range(B)], axis=1)).astype(FP8NP)
    wf_b = np.ascontiguousarray(np.asarray(Wf, np.float32)).astype(BF16NP)

    in_maps = []
    for c in range(N_CORES):
        d0 = 128 * c
        in_maps.append(
            {
                "xq": xq,
                "xk": xk,
                "xv": xv,
                "xresfc": np.ascontiguousarray(
                    q_[c // 4].T[:, 512 * (c % 4) : 512 * (c % 4 + 1)]
                ).astype(BF16NP),
                "wq": np.ascontiguousarray(np.asarray(Wq, np.float32)[:, d0 : d0 + 128] * WSCALE).astype(FP8NP),
                "wk": np.ascontiguousarray(np.asarray(Wk, np.float32)[:, d0 : d0 + 128] * WSCALE).astype(FP8NP),
                "wv": np.ascontiguousarray(np.asarray(Wv, np.float32)[:, d0 : d0 + 128] * WSCALE).astype(FP8NP),
                "wf": wf_b,
                "bq": np.ascontiguousarray(np.asarray(bq, np.float32)[d0 : d0 + 128, None] * WSCALE),
                "bk": np.ascontiguousarray(np.asarray(bk, np.float32)[d0 : d0 + 128, None] * WSCALE),
                "bv": np.ascontiguousarray(np.asarray(bv, np.float32)[None, d0 : d0 + 128] * WSCALE).astype(BF16NP),
            }
        )

    if "nc" not in _CACHE:
        _CACHE["nc"] = _build()
    res = run_bass_kernel_spmd(_CACHE["nc"], in_maps, core_ids=list(range(N_CORES)))

    out = np.empty((B, S, D), np.float32)
    for c in range(N_CORES):
        y = res.results[c]["out"]
        out[c // 4, 512 * (c % 4) : 512 * (c % 4 + 1), :] = y
    out += np.asarray(bf, np.float32)[None, None, :]
    return out


if __name__ == "__main__":
    # smoke test with small random data through the numpy fallback shapes
    rng = np.random.default_rng(0)
    args = dict(
        q_=rng.standard_normal((B, S, D), dtype=np.float32),
        k_=rng.standard_normal((B, S, D), dtype=np.float32),
        v_=rng.standard_normal((B, S, D), dtype=np.float32),
        mask=np.ones((B, S), bool),
        Wq=rng.standard_normal((D, D), dtype=np.float32) * 0.02,
        bq=np.zeros(D, np.float32),
        Wk=rng.standard_normal((D, D), dtype=np.float32) * 0.02,
        bk=np.zeros(D, np.float32),
        Wv=rng.standard_normal((D, D), dtype=np.float32) * 0.02,
        bv=np.zeros(D, np.float32),
        Wf=rng.standard_normal((D, D), dtype=np.float32) * 0.02,
        bf=np.zeros(D, np.float32),
    )
    got = kernel(**args)
    want = _numpy_reference(**args)
    rel = np.abs(got - want).max() / np.abs(want).max()
    print("rel_err:", rel)
